# revision 13
# baseline (speedup 1.0000x reference)
"""Trainium2 Bass kernel for CustomISTFT (N_FFT=4096, HOP=1024, T=4096 frames).

Per core (frames sharded 512/core across 8 cores), no DRAM round trips:
  Stage 1 (contract j1): z rows gathered by residue mod 64 (batched strided
  DMAs); per j2 a [K<=128, 128] matmul; psum copied (bf16) to a slot tile,
  then an SBUF->SBUF fan-in DMA lands it in At2 [128(2*j2+r), 64(m1), T].
  Stage 2 (contract j2): lhsT = w2[:, m1, :] (trig * window * 4096/3, rows
  (j2,r)-interleaved, columns q-permuted: q<32 -> m2=2q, q>=32 -> m2=2q+1),
  rhs = At2[:, m1, :] -> xps [64(q), T]; copied to a staging tile and
  fan-in-DMA'd to x2[m1 + 64*par, ih, t] = x[t, n = 128*ih + p].
  OLA on vector/gpsimd with free-dim shifts only:
  sig[p, ih, s] = sum_r x2[p, ih + 8r, s - r]  -> outre [128, 8, SPAD] f32.
  The imaginary channel is exactly rank-2 (bins 0 and 2048) and is computed
  on the host, as is the halo-add between neighbor cores and the exact wsum
  correction on the two edge blocks.
"""

import numpy as np
import ml_dtypes

N_FFT = 4096
HOP = 1024
FREQ = 2049
T_FRAMES = 4096
N_CORES = 8
T_CORE = T_FRAMES // N_CORES  # 512
L_FULL = (T_FRAMES - 1) * HOP + N_FFT
OUT_LEN = L_FULL - N_FFT

_bf16 = ml_dtypes.bfloat16


# ---------------------------------------------------------------- weights
def canonical_rows(j2):
    """(c, k) input rows consumed by the stage-1 call group of column j2."""
    if j2 == 0:
        return [(0, 64 * j1) for j1 in range(33)] + [(1, 64 * j1) for j1 in range(33)]
    if j2 == 32:
        return [(0, 32 + 64 * j1) for j1 in range(32)] + [
            (1, 32 + 64 * j1) for j1 in range(32)
        ]
    if j2 > 32:
        return canonical_rows(64 - j2)
    return (
        [(0, j2 + 64 * j1) for j1 in range(32)]
        + [(1, j2 + 64 * j1) for j1 in range(32)]
        + [(0, (64 - j2) + 64 * j1) for j1 in range(32)]
        + [(1, (64 - j2) + 64 * j1) for j1 in range(32)]
    )


def build_weights(window):
    """w1 [128,64,128] bf16 ([k, j2, (r,m1)], rows follow canonical_rows),
    w2 [128,64,64] bf16 ([row=(2*j2+r), m1, q], window*4096/3 folded)."""
    win = window.astype(np.float64)
    mu = np.exp(2j * np.pi / 4096)
    w64c = np.exp(2j * np.pi / 64)
    m1v = np.arange(64)

    w1 = np.zeros((64, 128, 128), dtype=np.float32)
    for j2 in range(64):
        coef = {}
        for j1 in range(64):
            k = 64 * j1 + j2
            e = w64c ** (m1v * j1)
            if k <= 2048:
                coef[(0, k)] = coef.get((0, k), 0) + e
                coef[(1, k)] = coef.get((1, k), 0) + 1j * e
            else:
                kr = 4096 - k
                coef[(0, kr)] = coef.get((0, kr), 0) + e
                coef[(1, kr)] = coef.get((1, kr), 0) - 1j * e
        tw = mu ** (m1v * j2)
        rows = canonical_rows(j2)
        assert set(rows) == set(coef.keys())
        for i, key in enumerate(rows):
            v = coef[key] * tw
            w1[j2, i, :64] = v.real.astype(np.float32)
            w1[j2, i, 64:] = v.imag.astype(np.float32)

    # w2[(2*j2+r), m1, q]: x[m1+64*m2] = sum_j2 A_re*cos - A_im*sin, * win/3.
    # Columns q permuted: q<32 -> m2=2q (even), q>=32 -> m2=2(q-32)+1 (odd).
    m2v = np.arange(64)
    j2v = np.arange(64)
    ang = 2 * np.pi * np.outer(j2v, m2v) / 64
    c = np.cos(ang) / 4096
    s = np.sin(ang) / 4096
    qperm = np.concatenate([2 * np.arange(32), 2 * np.arange(32) + 1])  # m2(q)
    w2 = np.zeros((128, 64, 64), dtype=np.float64)
    for m1 in range(64):
        n = m1 + 64 * m2v
        wn = win[n] * (4096.0 / 3.0)
        w2[0::2, m1, :] = (c * wn[None, :])[:, qperm]
        w2[1::2, m1, :] = (-s * wn[None, :])[:, qperm]

    w1k = np.ascontiguousarray(w1.astype(_bf16).transpose(1, 0, 2))
    return w1k, w2.astype(_bf16)


# ---------------------------------------------------------------- device program
def emit_kernel(tc, outre_ap, z_ap, w1_ap, w2_ap, T):
    """Per-core program.  T frames (multiple of 128).
    outre [128, 8, SPAD] f32: outre[p, ih, s] = overlap-added (windowed,
    *4096/3 normalized) real-channel signal at sample 1024*s + 128*ih + p."""
    import concourse.mybir as mybir
    from contextlib import ExitStack

    nc = tc.nc
    dt = mybir.dt
    f32, bf16 = dt.float32, dt.bfloat16
    SB = T + 3
    assert outre_ap.shape[2] >= SB

    rings = [nc.sync, nc.scalar]
    nring = [0]

    def ring():
        nring[0] += 1
        return rings[nring[0] % 2]

    # gpsimd cannot touch PSUM: PSUM->SBUF copies go to nc.any (vector/
    # scalar); SBUF->SBUF work alternates vector/gpsimd.
    cps = [nc.vector, nc.gpsimd]

    with ExitStack() as ctx:
        const = ctx.enter_context(tc.tile_pool(name="const", bufs=1))

        # ---- weights to SBUF (contiguous)
        w1_sb = const.tile([128, 64, 128], bf16)
        nc.scalar.dma_start(w1_sb[:], w1_ap[:])
        w2_sb = const.tile([128, 64, 64], bf16)
        nc.scalar.dma_start(w2_sb[:], w2_ap[:])

        # ---- persistent intermediates
        at2 = const.tile([128, 64, T], bf16)  # [(2*j2+r), m1, t]
        x2 = const.tile([128, 32, T], bf16)  # [p, ih, t] = x[t, 128*ih+p]

        zpool = ctx.enter_context(tc.tile_pool(name="zt", bufs=3))
        s1ps = ctx.enter_context(tc.tile_pool(name="s1ps", bufs=3, space="PSUM"))
        aspool = ctx.enter_context(tc.tile_pool(name="aslot", bufs=4))

        # ---- stage 1: gather z rows, matmul, fan-in corner DMA into at2
        zcnt = [0]

        def gather_z(p):
            K = 66 if p == 0 else (64 if p == 32 else 128)
            zt = zpool.tile([128, T], f32, tag="zt")
            if p == 0:
                nc.sync.dma_start(zt[0:66, :], z_ap[:, 0::64, :][:, 0:33, :])
            elif p == 32:
                nc.scalar.dma_start(zt[0:64, :], z_ap[:, 32::64, :][:, 0:32, :])
            else:
                ring().dma_start(zt[0:64, :], z_ap[:, p::64, :][:, 0:32, :])
                ring().dma_start(
                    zt[64:128, :], z_ap[:, (64 - p) :: 64, :][:, 0:32, :]
                )
            ztb = zpool.tile([128, T], bf16, tag="ztb")
            zcnt[0] += 1
            cps[zcnt[0] % 2].tensor_copy(ztb[0:K, :], zt[0:K, :])
            return ztb, K

        def s1_call(ztb, K, j2):
            ps = s1ps.tile([128, T], f32, tag="s1ps")
            nc.tensor.matmul(
                ps[:], w1_sb[0:K, j2, :], ztb[0:K, :], start=True, stop=True
            )
            asl = aspool.tile([128, T], bf16, tag="aslot")
            nc.any.tensor_copy(asl[:], ps[:])
            ring().dma_start(at2[2 * j2 : 2 * j2 + 2, :, :], asl[:])

        for p in range(33):
            ztb, K = gather_z(p)
            for j2 in ((p,) if p in (0, 32) else (p, 64 - p)):
                s1_call(ztb, K, j2)

        # ---- stage 2 + fan-in corner DMA into x2
        s2ps = ctx.enter_context(tc.tile_pool(name="s2ps", bufs=3, space="PSUM"))
        xpool = ctx.enter_context(tc.tile_pool(name="xq", bufs=2))
        XC = 8
        for mc in range(0, 64, XC):
            xq = xpool.tile([64, XC, T], bf16, tag="xq")
            for d in range(XC):
                m1 = mc + d
                xps = s2ps.tile([64, T], f32, tag="s2ps")
                nc.tensor.matmul(
                    xps[:], w2_sb[:, m1, :], at2[:, m1, :], start=True, stop=True
                )
                nc.any.tensor_copy(xq[:, d, :], xps[:])
            for d in range(XC):
                m1 = mc + d
                ring().dma_start(x2[m1 : m1 + 1, :, :], xq[0:32, d, :])
                ring().dma_start(x2[64 + m1 : 65 + m1, :, :], xq[32:64, d, :])

        # ---- OLA (free-dim shifts): sig[p, ih, s] = sum_r x2[p, ih+8r, s-r]
        SPAD = outre_ap.shape[2]
        sig = const.tile([128, 8, SPAD], f32)
        nc.vector.memset(sig[:, :, T:SPAD], 0.0)
        for half in range(2):
            eng = cps[half]
            o = 4 * half
            eng.tensor_copy(sig[:, o : o + 4, 0:T], x2[:, o : o + 4, :])
            for r in range(1, 4):
                eng.tensor_add(
                    sig[:, o : o + 4, r : r + T],
                    sig[:, o : o + 4, r : r + T],
                    x2[:, 8 * r + o : 8 * r + o + 4, :],
                )
        nc.scalar.dma_start(outre_ap[:], sig[:])


# ---------------------------------------------------------------- build + run
_CACHE = {}
SPAD = 520  # padded s extent of outre (>= T_CORE + 3)


def _build(T):
    import concourse.bacc as bacc
    import concourse.tile as tile
    import concourse.mybir as mybir

    dt = mybir.dt
    nc = bacc.Bacc("TRN2", target_bir_lowering=False, debug=False, num_devices=N_CORES)
    z_t = nc.dram_tensor("z", [2, FREQ, T], dt.float32, kind="ExternalInput")
    w1_t = nc.dram_tensor("w1", [128, 64, 128], dt.bfloat16, kind="ExternalInput")
    w2_t = nc.dram_tensor("w2", [128, 64, 64], dt.bfloat16, kind="ExternalInput")
    spad = max(SPAD, T + 3)
    outre_t = nc.dram_tensor("outre", [128, 8, spad], dt.float32, kind="ExternalOutput")
    with tile.TileContext(nc) as tc:
        emit_kernel(tc, outre_t.ap(), z_t.ap(), w1_t.ap(), w2_t.ap(), T)
    nc.compile()
    return nc


def _host_imag(z, window):
    """Imag channel of the full output: rank-2 in (z[1,0,:], z[1,2048,:]).
    imag(frame_t)[n] = (b0[t] + (-1)^n b2048[t]) / 4096; windowed OLA, then
    the same wsum normalization as the real channel."""
    win = window.astype(np.float64)
    b0 = z[1, 0, :].astype(np.float64)
    b2 = z[1, 2048, :].astype(np.float64)
    T = z.shape[2]
    nblk = T + 3  # output blocks of 1024 before crop
    # im[u0, i], u = 1024*u0 + i: sum_r win[i + 1024 r] * c[u0 - r]
    sgn = np.where(np.arange(1024) % 2 == 0, 1.0, -1.0)
    im = np.zeros((nblk, 1024))
    for r in range(4):
        wv = win[1024 * r : 1024 * r + 1024] / 4096.0
        t0 = np.arange(nblk) - r
        valid = (t0 >= 0) & (t0 < T)
        cc = np.where(valid, b0[np.clip(t0, 0, T - 1)], 0.0)
        dd = np.where(valid, b2[np.clip(t0, 0, T - 1)], 0.0)
        im += np.outer(cc, wv) + np.outer(dd, wv * sgn)
    return im.reshape(-1)  # [ (T+3)*1024 ], needs wsum normalization


def kernel(z, window):
    from concourse.bass_utils import run_bass_kernel_spmd

    z = np.asarray(z, dtype=np.float32)
    window = np.asarray(window, dtype=np.float32)
    assert z.shape == (2, FREQ, T_FRAMES)

    if "nc" not in _CACHE:
        _CACHE["nc"] = _build(T_CORE)
    nc = _CACHE["nc"]

    w1, w2 = build_weights(window)
    in_maps = []
    for m in range(N_CORES):
        zc = np.ascontiguousarray(z[:, :, m * T_CORE : (m + 1) * T_CORE])
        in_maps.append({"z": zc, "w1": w1, "w2": w2})
    res = run_bass_kernel_spmd(nc, in_maps, core_ids=list(range(N_CORES)))

    # device sig is interior-normalized (win * 4096/3 folded into w2)
    full = np.zeros((2, L_FULL), dtype=np.float64)
    span = (T_CORE + 3) * 1024
    for m in range(N_CORES):
        outre = res.results[m]["outre"]
        re = outre.transpose(2, 1, 0).reshape(-1, 1024)[: T_CORE + 3]
        full[0, m * T_CORE * HOP : m * T_CORE * HOP + span] += re.reshape(-1)

    # imag channel (rank-2) on host; same interior normalization
    full[1, :] = _host_imag(z, window) * (4096.0 / 3.0)

    out = full[:, N_FFT // 2 : L_FULL - N_FFT // 2]

    # edge blocks: interior-normalized values rescaled by (3/4096)/wsum
    win = window.astype(np.float64)
    ws_start = win[0:1024] + win[1024:2048] + win[2048:3072]
    ws_end = win[1024:2048] + win[2048:3072] + win[3072:4096]
    out[:, :1024] *= ((3.0 / 4096.0) / ws_start)[None, :]
    out[:, -1024:] *= ((3.0 / 4096.0) / ws_end)[None, :]
    return np.ascontiguousarray(out.astype(np.float32))


# revision 16
# speedup vs baseline: 1.4932x; 1.4932x over previous
"""Trainium2 Bass kernel for CustomISTFT (N_FFT=4096, HOP=1024, T=4096 frames).

Per core (frames sharded 512/core across 8 cores), no DRAM round trips:
  Stage 1 (contract j1): z rows gathered by residue mod 64 (batched strided
  DMAs); per j2 a [K<=128, 128] matmul; psum copied (bf16) to a slot tile,
  then an SBUF->SBUF fan-in DMA lands it in At2 [128(2*j2+r), 64(m1), T].
  Stage 2 (contract j2): lhsT = w2[:, m1, :] (trig * window * 4096/3, rows
  (j2,r)-interleaved, columns q-permuted: q<32 -> m2=2q, q>=32 -> m2=2q+1),
  rhs = At2[:, m1, :] -> xps [64(q), T]; copied to a staging tile and
  fan-in-DMA'd to x2[m1 + 64*par, ih, t] = x[t, n = 128*ih + p].
  OLA on vector/gpsimd with free-dim shifts only:
  sig[p, ih, s] = sum_r x2[p, ih + 8r, s - r]  -> outre [128, 8, SPAD] f32.
  The imaginary channel is exactly rank-2 (bins 0 and 2048) and is computed
  on the host, as is the halo-add between neighbor cores and the exact wsum
  correction on the two edge blocks.
"""

import numpy as np
import ml_dtypes

N_FFT = 4096
HOP = 1024
FREQ = 2049
T_FRAMES = 4096
N_CORES = 8
T_CORE = T_FRAMES // N_CORES  # 512
L_FULL = (T_FRAMES - 1) * HOP + N_FFT
OUT_LEN = L_FULL - N_FFT

_bf16 = ml_dtypes.bfloat16

# Store the stage-1 intermediate (at2) as fp8 e3m4 with a 1/8 scale folded
# into w1 (w2 carries the compensating 8x): halves the corner-turn bytes.
# Sim rel err 0.0137 (vs 0.0035 at bf16); budget is 2e-2.
AT2_FP8 = True


# ---------------------------------------------------------------- weights
def canonical_rows(j2):
    """(c, k) input rows consumed by the stage-1 call group of column j2."""
    if j2 == 0:
        return [(0, 64 * j1) for j1 in range(33)] + [(1, 64 * j1) for j1 in range(33)]
    if j2 == 32:
        return [(0, 32 + 64 * j1) for j1 in range(32)] + [
            (1, 32 + 64 * j1) for j1 in range(32)
        ]
    if j2 > 32:
        return canonical_rows(64 - j2)
    return (
        [(0, j2 + 64 * j1) for j1 in range(32)]
        + [(1, j2 + 64 * j1) for j1 in range(32)]
        + [(0, (64 - j2) + 64 * j1) for j1 in range(32)]
        + [(1, (64 - j2) + 64 * j1) for j1 in range(32)]
    )


def build_weights(window):
    """w1 [128,64,128] bf16 ([k, j2, (r,m1)], rows follow canonical_rows),
    w2 [128,64,64] bf16 ([row=(2*j2+r), m1, q], window*4096/3 folded)."""
    win = window.astype(np.float64)
    mu = np.exp(2j * np.pi / 4096)
    w64c = np.exp(2j * np.pi / 64)
    m1v = np.arange(64)

    w1 = np.zeros((64, 128, 128), dtype=np.float32)
    for j2 in range(64):
        coef = {}
        for j1 in range(64):
            k = 64 * j1 + j2
            e = w64c ** (m1v * j1)
            if k <= 2048:
                coef[(0, k)] = coef.get((0, k), 0) + e
                coef[(1, k)] = coef.get((1, k), 0) + 1j * e
            else:
                kr = 4096 - k
                coef[(0, kr)] = coef.get((0, kr), 0) + e
                coef[(1, kr)] = coef.get((1, kr), 0) - 1j * e
        tw = mu ** (m1v * j2)
        rows = canonical_rows(j2)
        assert set(rows) == set(coef.keys())
        for i, key in enumerate(rows):
            v = coef[key] * tw
            w1[j2, i, :64] = v.real.astype(np.float32)
            w1[j2, i, 64:] = v.imag.astype(np.float32)

    # w2[(2*j2+r), m1, q]: x[m1+64*m2] = sum_j2 A_re*cos - A_im*sin, * win/3.
    # Columns q permuted: q<32 -> m2=2q (even), q>=32 -> m2=2(q-32)+1 (odd).
    m2v = np.arange(64)
    j2v = np.arange(64)
    ang = 2 * np.pi * np.outer(j2v, m2v) / 64
    c = np.cos(ang) / 4096
    s = np.sin(ang) / 4096
    qperm = np.concatenate([2 * np.arange(32), 2 * np.arange(32) + 1])  # m2(q)
    w2 = np.zeros((128, 64, 64), dtype=np.float64)
    for m1 in range(64):
        n = m1 + 64 * m2v
        wn = win[n] * (4096.0 / 3.0)
        w2[0::2, m1, :] = (c * wn[None, :])[:, qperm]
        w2[1::2, m1, :] = (-s * wn[None, :])[:, qperm]

    if AT2_FP8:
        w1 = w1 / 8.0
        w2 = w2 * 8.0
    w1k = np.ascontiguousarray(w1.astype(_bf16).transpose(1, 0, 2))
    return w1k, w2.astype(_bf16)


# ---------------------------------------------------------------- device program
def emit_kernel(tc, outre_ap, z_ap, w1_ap, w2_ap, T):
    """Per-core program.  T frames (multiple of 128).
    outre [128, 8, SPAD] f32: outre[p, ih, s] = overlap-added (windowed,
    *4096/3 normalized) real-channel signal at sample 1024*s + 128*ih + p."""
    import concourse.mybir as mybir
    from contextlib import ExitStack

    nc = tc.nc
    dt = mybir.dt
    f32, bf16 = dt.float32, dt.bfloat16
    SB = T + 3
    assert outre_ap.shape[2] >= SB

    rings = [nc.sync, nc.scalar]
    nring = [0]

    def ring():
        nring[0] += 1
        return rings[nring[0] % 2]

    # gpsimd cannot touch PSUM: PSUM->SBUF copies go to nc.any (vector/
    # scalar); SBUF->SBUF work alternates vector/gpsimd.
    cps = [nc.vector, nc.gpsimd]

    at2dt = dt.float8e3 if AT2_FP8 else bf16

    with ExitStack() as ctx:
        const = ctx.enter_context(tc.tile_pool(name="const", bufs=1))

        # ---- weights to SBUF (contiguous)
        w1_sb = const.tile([128, 64, 128], bf16)
        nc.scalar.dma_start(w1_sb[:], w1_ap[:])
        w2_sb = const.tile([128, 64, 64], bf16)
        nc.scalar.dma_start(w2_sb[:], w2_ap[:])

        # ---- corner turns go through DRAM in transposed layout: the DRAM AP
        # absorbs the permutation, both SBUF sides stay partition-balanced,
        # and the read-back is contiguous.
        dram = ctx.enter_context(tc.tile_pool(name="dram", bufs=1, space="DRAM"))
        at2_dram = dram.tile([128, 64, T], at2dt)  # [(2*j2+r), m1, t]
        x_dram = dram.tile([128, 32, T], bf16)  # [p, ih, t] = x[t, 128*ih+p]

        # ---- persistent intermediates
        at2 = const.tile([128, 64, T], at2dt)
        x2 = const.tile([128, 32, T], bf16)

        zpool = ctx.enter_context(tc.tile_pool(name="zt", bufs=3))
        s1ps = ctx.enter_context(tc.tile_pool(name="s1ps", bufs=3, space="PSUM"))
        aspool = ctx.enter_context(tc.tile_pool(name="aslot", bufs=4))

        # ---- stage 1: gather z rows, matmul, corner write to at2_dram
        zcnt = [0]

        def gather_z(p):
            K = 66 if p == 0 else (64 if p == 32 else 128)
            zt = zpool.tile([128, T], f32, tag="zt")
            if p == 0:
                nc.sync.dma_start(zt[0:66, :], z_ap[:, 0::64, :][:, 0:33, :])
            elif p == 32:
                nc.scalar.dma_start(zt[0:64, :], z_ap[:, 32::64, :][:, 0:32, :])
            else:
                ring().dma_start(zt[0:64, :], z_ap[:, p::64, :][:, 0:32, :])
                ring().dma_start(
                    zt[64:128, :], z_ap[:, (64 - p) :: 64, :][:, 0:32, :]
                )
            ztb = zpool.tile([128, T], bf16, tag="ztb")
            zcnt[0] += 1
            cps[zcnt[0] % 2].tensor_copy(ztb[0:K, :], zt[0:K, :])
            return ztb, K

        def s1_call(ztb, K, j2):
            ps = s1ps.tile([128, T], f32, tag="s1ps")
            nc.tensor.matmul(
                ps[:], w1_sb[0:K, j2, :], ztb[0:K, :], start=True, stop=True
            )
            asl = aspool.tile([128, T], at2dt, tag="aslot")
            nc.any.tensor_copy(asl[:], ps[:])
            ring().dma_start(at2_dram[2 * j2 : 2 * j2 + 2, :, :], asl[:])

        for p in range(33):
            ztb, K = gather_z(p)
            for j2 in ((p,) if p in (0, 32) else (p, 64 - p)):
                s1_call(ztb, K, j2)

        # ---- stage 2 (at2 read back per chunk) + corner write to x_dram
        s2ps = ctx.enter_context(tc.tile_pool(name="s2ps", bufs=3, space="PSUM"))
        xpool = ctx.enter_context(tc.tile_pool(name="xq", bufs=2))
        XC = 8
        for mc in range(0, 64, XC):
            ring().dma_start(at2[:, mc : mc + XC, :], at2_dram[:, mc : mc + XC, :])
            xq = xpool.tile([64, XC, T], bf16, tag="xq")
            for d in range(XC):
                m1 = mc + d
                xps = s2ps.tile([64, T], f32, tag="s2ps")
                nc.tensor.matmul(
                    xps[:], w2_sb[:, m1, :], at2[:, m1, :], start=True, stop=True
                )
                nc.any.tensor_copy(xq[:, d, :], xps[:])
            ring().dma_start(
                x_dram[mc : mc + XC, :, :].rearrange("d ih t -> ih d t"),
                xq[0:32, :, :],
            )
            ring().dma_start(
                x_dram[64 + mc : 64 + mc + XC, :, :].rearrange("d ih t -> ih d t"),
                xq[32:64, :, :],
            )
        nc.sync.dma_start(x2[:], x_dram[:])

        # ---- OLA (free-dim shifts): sig[p, ih, s] = sum_r x2[p, ih+8r, s-r]
        SPAD = outre_ap.shape[2]
        sig = const.tile([128, 8, SPAD], f32)
        nc.vector.memset(sig[:, :, T:SPAD], 0.0)
        for half in range(2):
            eng = cps[half]
            o = 4 * half
            eng.tensor_copy(sig[:, o : o + 4, 0:T], x2[:, o : o + 4, :])
            for r in range(1, 4):
                eng.tensor_add(
                    sig[:, o : o + 4, r : r + T],
                    sig[:, o : o + 4, r : r + T],
                    x2[:, 8 * r + o : 8 * r + o + 4, :],
                )
        nc.scalar.dma_start(outre_ap[:], sig[:])


# ---------------------------------------------------------------- build + run
_CACHE = {}
SPAD = 520  # padded s extent of outre (>= T_CORE + 3)


def _build(T):
    import concourse.bacc as bacc
    import concourse.tile as tile
    import concourse.mybir as mybir

    dt = mybir.dt
    nc = bacc.Bacc("TRN2", target_bir_lowering=False, debug=False, num_devices=N_CORES)
    z_t = nc.dram_tensor("z", [2, FREQ, T], dt.float32, kind="ExternalInput")
    w1_t = nc.dram_tensor("w1", [128, 64, 128], dt.bfloat16, kind="ExternalInput")
    w2_t = nc.dram_tensor("w2", [128, 64, 64], dt.bfloat16, kind="ExternalInput")
    spad = max(SPAD, T + 3)
    outre_t = nc.dram_tensor("outre", [128, 8, spad], dt.float32, kind="ExternalOutput")
    with tile.TileContext(nc) as tc:
        emit_kernel(tc, outre_t.ap(), z_t.ap(), w1_t.ap(), w2_t.ap(), T)
    nc.compile()
    return nc


def _host_imag(z, window):
    """Imag channel of the full output: rank-2 in (z[1,0,:], z[1,2048,:]).
    imag(frame_t)[n] = (b0[t] + (-1)^n b2048[t]) / 4096; windowed OLA, then
    the same wsum normalization as the real channel."""
    win = window.astype(np.float64)
    b0 = z[1, 0, :].astype(np.float64)
    b2 = z[1, 2048, :].astype(np.float64)
    T = z.shape[2]
    nblk = T + 3  # output blocks of 1024 before crop
    # im[u0, i], u = 1024*u0 + i: sum_r win[i + 1024 r] * c[u0 - r]
    sgn = np.where(np.arange(1024) % 2 == 0, 1.0, -1.0)
    im = np.zeros((nblk, 1024))
    for r in range(4):
        wv = win[1024 * r : 1024 * r + 1024] / 4096.0
        t0 = np.arange(nblk) - r
        valid = (t0 >= 0) & (t0 < T)
        cc = np.where(valid, b0[np.clip(t0, 0, T - 1)], 0.0)
        dd = np.where(valid, b2[np.clip(t0, 0, T - 1)], 0.0)
        im += np.outer(cc, wv) + np.outer(dd, wv * sgn)
    return im.reshape(-1)  # [ (T+3)*1024 ], needs wsum normalization


def kernel(z, window):
    from concourse.bass_utils import run_bass_kernel_spmd

    z = np.asarray(z, dtype=np.float32)
    window = np.asarray(window, dtype=np.float32)
    assert z.shape == (2, FREQ, T_FRAMES)

    if "nc" not in _CACHE:
        _CACHE["nc"] = _build(T_CORE)
    nc = _CACHE["nc"]

    w1, w2 = build_weights(window)
    in_maps = []
    for m in range(N_CORES):
        zc = np.ascontiguousarray(z[:, :, m * T_CORE : (m + 1) * T_CORE])
        in_maps.append({"z": zc, "w1": w1, "w2": w2})
    res = run_bass_kernel_spmd(nc, in_maps, core_ids=list(range(N_CORES)))

    # device sig is interior-normalized (win * 4096/3 folded into w2)
    full = np.zeros((2, L_FULL), dtype=np.float64)
    span = (T_CORE + 3) * 1024
    for m in range(N_CORES):
        outre = res.results[m]["outre"]
        re = outre.transpose(2, 1, 0).reshape(-1, 1024)[: T_CORE + 3]
        full[0, m * T_CORE * HOP : m * T_CORE * HOP + span] += re.reshape(-1)

    # imag channel (rank-2) on host; same interior normalization
    full[1, :] = _host_imag(z, window) * (4096.0 / 3.0)

    out = full[:, N_FFT // 2 : L_FULL - N_FFT // 2]

    # edge blocks: interior-normalized values rescaled by (3/4096)/wsum
    win = window.astype(np.float64)
    ws_start = win[0:1024] + win[1024:2048] + win[2048:3072]
    ws_end = win[1024:2048] + win[2048:3072] + win[3072:4096]
    out[:, :1024] *= ((3.0 / 4096.0) / ws_start)[None, :]
    out[:, -1024:] *= ((3.0 / 4096.0) / ws_end)[None, :]
    return np.ascontiguousarray(out.astype(np.float32))
